# revision 10
# baseline (speedup 1.0000x reference)
"""Trainium2 Bass kernel for per-pixel dot-product attention.

Reference op (per pixel, over C=80 channels split q/k/v = 8/64/8):
    qk[v] = sum_k q[k] * K[k, v] / sqrt(8)
    attn  = softmax(qk over v)
    out[v] = attn[v] * V[v]

Strategy: pure data-parallel over 8 NeuronCores — core i handles batch
i//2, H-rows half (i%2).  The per-core shard is pre-transposed on the
HOST to a partition-major layout [128, C * 1024]: partition p owns
pixels [p*1024, (p+1)*1024), and the free dim is a concatenation of
per-chunk [C x ncol] channel-major blocks.  Each chunk then loads with
ONE HWDGE dma_start whose descriptors are C*ncol*4 (~40-50 KB)
contiguous bytes per partition — line-rate DMA (the v1 layout paid 1 KB
descriptors: ~21 GB/s/engine = 339 GB/s aggregate, vs ~27 GB/s/engine
asymptotic).  ScalarE converts K and q to bf16 (and does the exp), DVE
does all multiplies and the pairwise add-trees at bf16 2x rate, and the
final multiply writes a bf16 output tile that stores to a bf16 y (host
upcasts) — halving output HBM bytes; rel-l2 err ~5e-3 vs the 2e-2 gate.
"""

import numpy as np

NK = 8
NV = 8
C = NK + NK * NV + NV  # 80
B, H, W = 4, 512, 512
N_CORES = 8
ROWS = H // 2            # rows per core
PIX = ROWS * W           # pixels per core (131072)
XCOLS = PIX // 128       # free-dim pixels per partition (1024)
_SCALE = 1.0 / float(np.sqrt(NK))

# per-chunk free-dim widths; big head chunks for DMA efficiency, tapered
# tail so the post-prod serial chain (softmax/out) drains fast
CHUNKS = [160, 160, 160, 160, 160, 128, 48, 32, 16]
assert sum(CHUNKS) == XCOLS


def _ensure_path():
    import sys
    p = "/opt/trn_rl_repo"
    if p not in sys.path:
        sys.path.insert(0, p)


def build_nc(chunk_cols=None, in_bufs=3, e_bufs=2, o_bufs=2,
             acc_bufs=2, piece_bufs=3, recip_on_act=False):
    """Per-core Bass program: x [128, C*XCOLS] f32 -> y [128, NV*XCOLS] bf16.

    One input dma_start per chunk (sync ring), one bf16 output store per
    chunk (scalar ring, deferred behind the next chunk's load trigger).
    The k-reduction is a serial accumulator — each converted K piece is
    multiplied by q and immediately added into acc — so qk (= acc) is
    ready one add after the last multiply and the inline exp stalls ACT
    only ~1 us.  Rotating piece buffers keep the conversions streaming.
    """
    _ensure_path()
    import concourse.tile as tile
    from concourse import bacc, mybir

    f32 = mybir.dt.float32
    bf16 = mybir.dt.bfloat16
    if chunk_cols is None:
        chunk_cols = CHUNKS
    assert sum(chunk_cols) == XCOLS

    nc = bacc.Bacc("TRN2", target_bir_lowering=False, debug=False)
    x = nc.dram_tensor("x", [128, C * XCOLS], f32, kind="ExternalInput")
    y = nc.dram_tensor("y", [128, NV * XCOLS], bf16, kind="ExternalOutput")

    pending_out = []

    def flush_out():
        for args in pending_out:
            nc.scalar.dma_start(**args)
        pending_out.clear()

    with tile.TileContext(nc) as tc:
        with (
            tc.tile_pool(name="inp", bufs=1) as in_pool,
            tc.tile_pool(name="work", bufs=1) as work_pool,
            tc.tile_pool(name="pipe", bufs=1) as pipe_pool,
        ):
            def emit_softmax(st):
                """exp + v-sum + reciprocal + output multiplies for a chunk
                whose accumulator is complete.  Called one piece into the
                NEXT chunk's product stream so the exp overlaps DVE's first
                multiply and the DVE ops interleave with later multiplies."""
                j, n, off, it, acc = st
                e = pipe_pool.tile([128, NV * n], bf16, name=f"e{j}", tag="e",
                                   bufs=e_bufs)
                nc.scalar.activation(e, acc, mybir.ActivationFunctionType.Exp,
                                     scale=_SCALE)
                t1 = pipe_pool.tile([128, 4 * n], bf16, name=f"t1_{j}",
                                    tag="t1", bufs=1)
                nc.vector.tensor_tensor(t1, e[:, 0:4 * n], e[:, 4 * n:],
                                        mybir.AluOpType.add)
                nc.vector.tensor_tensor(t1[:, 0:2 * n], t1[:, 0:2 * n],
                                        t1[:, 2 * n:], mybir.AluOpType.add)
                sc = pipe_pool.tile([128, 2 * n], f32, name=f"sc{j}",
                                    tag="sc", bufs=1)
                s = sc[:, 0:n]
                nc.vector.tensor_tensor(s, t1[:, 0:n], t1[:, n:2 * n],
                                        mybir.AluOpType.add)
                r = sc[:, n:2 * n]
                if recip_on_act:
                    ls = sc[:, 0:n]
                    nc.scalar.activation(ls, s,
                                         mybir.ActivationFunctionType.Ln)
                    nc.scalar.activation(r, ls,
                                         mybir.ActivationFunctionType.Exp,
                                         scale=-1.0)
                else:
                    nc.vector.reciprocal(r, s)

                # out[v] = e[v] * (V[v] * r): vr in bf16 (f32 inputs), then
                # one bf16 x bf16 multiply into the bf16 out tile.  The
                # stride-0 broadcast operand goes in in0 (half rate in in1).
                v3 = it.rearrange("p (c x) -> p c x", c=C)[:, NK + NK * NV:C]
                r_b = r.unsqueeze(1).broadcast_to((128, NV, n))
                vr = pipe_pool.tile([128, NV * n], bf16, name=f"vr{j}",
                                    tag="vr", bufs=1)
                vr3 = vr.rearrange("p (v x) -> p v x", v=NV)
                nc.vector.tensor_tensor(vr3, r_b, v3, mybir.AluOpType.mult)
                ob = pipe_pool.tile([128, NV * n], bf16, name=f"o{j}",
                                    tag="o", bufs=o_bufs)
                nc.vector.tensor_tensor(ob, vr, e, mybir.AluOpType.mult)
                pending_out.append(dict(
                    out=y[:, NV * off:NV * (off + n)], in_=ob,
                ))

            pending_sm = None
            off = 0
            for j, n in enumerate(chunk_cols):
                it = in_pool.tile([128, C * n], f32, name=f"in{j}", tag="in",
                                  bufs=in_bufs)
                nc.sync.dma_start(out=it, in_=x[:, C * off:C * (off + n)])
                # chunk j-2's store, behind this chunk's load trigger
                flush_out()

                q_bf = work_pool.tile([128, NK * n], bf16, name=f"qbf{j}",
                                      tag="qbf", bufs=2)
                nc.scalar.activation(q_bf, it[:, 0:NK * n],
                                     mybir.ActivationFunctionType.Copy)
                q_b = (
                    q_bf.rearrange("p (k x) -> p k x", k=NK)
                    .unsqueeze(2)
                    .broadcast_to((128, NK, NV, n))
                )

                # serial-accumulated products: acc = sum_k q[k] * K[k, :]
                acc = work_pool.tile([128, NV * n], bf16, name=f"acc{j}",
                                     tag="acc", bufs=acc_bufs)
                a4 = acc.rearrange("p (v x) -> p v x", v=NV).unsqueeze(1)
                for k in range(NK):
                    src = it[:, (NK + k * NV) * n:(NK + (k + 1) * NV) * n]
                    if k == 0:
                        nc.scalar.activation(
                            acc, src, mybir.ActivationFunctionType.Copy)
                        nc.vector.tensor_tensor(
                            a4, q_b[:, 0:1], a4, mybir.AluOpType.mult)
                        # previous chunk's softmax rides here: its exp runs
                        # on ACT while DVE does this chunk's first multiply,
                        # and its DVE tail interleaves with pieces 1-7
                        if pending_sm is not None:
                            emit_softmax(pending_sm)
                            pending_sm = None
                        continue
                    pc = work_pool.tile([128, NV * n], bf16, name=f"pc{j}_{k}",
                                        tag="pc", bufs=piece_bufs)
                    p4 = pc.rearrange("p (v x) -> p v x", v=NV).unsqueeze(1)
                    nc.scalar.activation(
                        pc, src, mybir.ActivationFunctionType.Copy)
                    nc.vector.tensor_tensor(
                        p4, q_b[:, k:k + 1], p4, mybir.AluOpType.mult)
                    nc.vector.tensor_tensor(acc, acc, pc,
                                            mybir.AluOpType.add)

                pending_sm = (j, n, off, it, acc)
                off += n
            emit_softmax(pending_sm)
            flush_out()
    nc.compile()
    return nc


_NC_CACHE = {}

BUILD_CFG = {}


def _get_nc(**cfg):
    cfg = {**BUILD_CFG, **cfg}
    key = tuple(sorted(
        (k, tuple(v) if isinstance(v, list) else v) for k, v in cfg.items()
    ))
    if key not in _NC_CACHE:
        _NC_CACHE[key] = build_nc(**cfg)
    return _NC_CACHE[key]


def make_in_maps(inp, chunk_cols=None):
    """Host-side shard + transpose to the partition-major chunked layout."""
    if chunk_cols is None:
        chunk_cols = CHUNKS
    in_maps = []
    for core in range(N_CORES):
        b, hh = core // 2, core % 2
        t3 = np.asarray(
            inp[b, :, hh * ROWS:(hh + 1) * ROWS, :], dtype=np.float32
        ).reshape(C, 128, XCOLS).transpose(1, 0, 2)  # [128, C, XCOLS]
        off = 0
        parts = []
        for n in chunk_cols:
            parts.append(np.ascontiguousarray(
                t3[:, :, off:off + n]).reshape(128, C * n))
            off += n
        in_maps.append({"x": np.ascontiguousarray(
            np.concatenate(parts, axis=1))})
    return in_maps


def assemble_out(results, chunk_cols=None):
    if chunk_cols is None:
        chunk_cols = CHUNKS
    out = np.empty((B, NV, H, W), np.float32)
    for core in range(N_CORES):
        b, hh = core // 2, core % 2
        r = np.asarray(results[core]["y"]).astype(np.float32)  # [128, NV*XCOLS]
        off = 0
        blocks = []
        for n in chunk_cols:
            blocks.append(r[:, NV * off:NV * (off + n)].reshape(128, NV, n))
            off += n
        img = np.concatenate(blocks, axis=2)          # [128, NV, XCOLS]
        out[b, :, hh * ROWS:(hh + 1) * ROWS, :] = (
            img.transpose(1, 0, 2).reshape(NV, ROWS, W)
        )
    return out


def run_spmd(inp, trace=False, build_cfg=None, **kwargs):
    """Run the SPMD kernel on 8 cores; returns (full_output, BassKernelResults)."""
    _ensure_path()
    from concourse.bass_utils import run_bass_kernel_spmd

    inp = np.asarray(inp)
    assert inp.shape == (B, C, H, W), inp.shape
    cfg = dict(build_cfg or {})
    chunk_cols = cfg.get("chunk_cols") or CHUNKS
    nc = _get_nc(**cfg)
    res = run_bass_kernel_spmd(
        nc, make_in_maps(inp, chunk_cols), list(range(N_CORES)),
        trace=trace, **kwargs
    )
    return assemble_out(res.results, chunk_cols), res


def kernel(inp):
    out, _ = run_spmd(inp, trace=False)
    return out
